# Initial kernel scaffold
#
"""Performer (linear) attention kernel for Trainium2, 8-core SPMD.

Math (per batch b, head h):
    q  = relu(query) + eps
    k  = (relu(key) + eps) * mask[:, None]
    kv = k^T @ v                  # [D, D]
    ks = sum_s k                  # [D]
    num = q @ kv                  # [S, D]
    den = q @ ks                  # [S]
    out = num / den[:, None]

Sharding: 64 (b,h) heads split across 8 cores, 8 heads each. No collectives.

Per-head device plan (S=4096, D=64, P=128 partitions):
  - q/k/v loaded as [128, 2048] tiles (partition p holds rows s=32p..32p+31,
    contiguous 8KB per partition -> line-rate DMA). Chunk c = free columns
    [c*64, (c+1)*64) = the 128 rows {32p + c}.
  - DVE: k_prep/q_prep = max(x,0)+eps in one fused tensor_scalar.
  - PE:  kv[64,64] += k_chunk^T @ v_chunk, and ksum col kv[:,64] += k_chunk^T @ ones
         (32 accumulating chunk pairs into one PSUM tile [64,65]).
  - PE:  q_prep chunks transposed ([128,64]->[64,128]) into PSUM, ACT copies
         them to qT [64,4096] in SBUF.
  - PE:  num chunk [128,65] = qT_chunk^T @ kv_ext  (col 64 = denominator).
  - DVE: reciprocal of denom cols, then num * recip (broadcast) -> out tile.
  - Store out tile [128, 2048] with the same layout (one 1MB DMA).
"""

import numpy as np

from concourse import bass, mybir
import concourse.tile as tile
from concourse.masks import make_identity
from concourse.bass_utils import run_bass_kernel_spmd

B, H, S, D = 4, 16, 4096, 64
N_CORES = 8
HEADS_PER_CORE = (B * H) // N_CORES  # 8
P = 128
NCHUNK = S // P  # 32
EPS = 0.001
FP32 = mybir.dt.float32

TRACE = False
LAST_EXEC_NS = None


def _build_nc(use_mask: bool) -> bass.Bass:
    nc = bass.Bass(trn_type="TRN2")

    q_d = nc.dram_tensor("query", [HEADS_PER_CORE, S, D], FP32, kind="ExternalInput")
    k_d = nc.dram_tensor("key", [HEADS_PER_CORE, S, D], FP32, kind="ExternalInput")
    v_d = nc.dram_tensor("value", [HEADS_PER_CORE, S, D], FP32, kind="ExternalInput")
    if use_mask:
        m_d = nc.dram_tensor("mask", [HEADS_PER_CORE, S], FP32, kind="ExternalInput")
    o_d = nc.dram_tensor("out", [HEADS_PER_CORE, S, D], FP32, kind="ExternalOutput")

    with tile.TileContext(nc) as tc:
        with (
            tc.tile_pool(name="const", bufs=1) as const_pool,
            tc.tile_pool(name="io", bufs=2) as io_pool,
            tc.tile_pool(name="work", bufs=2) as work_pool,
            tc.tile_pool(name="small", bufs=2) as small_pool,
            tc.tile_pool(name="kvps", bufs=2, space="PSUM") as kvps_pool,
            tc.tile_pool(name="trps", bufs=2, space="PSUM") as trps_pool,
            tc.tile_pool(name="nups", bufs=2, space="PSUM") as nups_pool,
        ):
            ones_col = const_pool.tile([P, 1], FP32)
            nc.vector.memset(ones_col[:], 1.0)
            identity = const_pool.tile([P, P], FP32)
            make_identity(nc, identity[:])

            for hd in range(HEADS_PER_CORE):
                k_tile = io_pool.tile([P, NCHUNK * D], FP32, name="k_tile")
                v_tile = io_pool.tile([P, NCHUNK * D], FP32, name="v_tile")
                q_tile = io_pool.tile([P, NCHUNK * D], FP32, name="q_tile")
                nc.sync.dma_start(
                    k_tile[:], k_d[hd].rearrange("(p n) d -> p (n d)", p=P)
                )
                nc.sync.dma_start(
                    v_tile[:], v_d[hd].rearrange("(p n) d -> p (n d)", p=P)
                )
                nc.sync.dma_start(
                    q_tile[:], q_d[hd].rearrange("(p n) d -> p (n d)", p=P)
                )
                if use_mask:
                    m_tile = small_pool.tile([P, NCHUNK], FP32, name="m_tile")
                    nc.sync.dma_start(
                        m_tile[:], m_d[hd].rearrange("(p n) -> p n", p=P)
                    )

                # k_prep = max(key, 0) + eps   (then * mask if present)
                k_prep = work_pool.tile([P, NCHUNK * D], FP32, name="k_prep")
                nc.vector.tensor_scalar(
                    out=k_prep[:],
                    in0=k_tile[:],
                    scalar1=0.0,
                    scalar2=EPS,
                    op0=mybir.AluOpType.max,
                    op1=mybir.AluOpType.add,
                )
                if use_mask:
                    nc.vector.tensor_tensor(
                        out=k_prep.rearrange("p (n d) -> p n d", d=D)[:],
                        in0=k_prep.rearrange("p (n d) -> p n d", d=D)[:],
                        in1=m_tile[:, :, None].to_broadcast([P, NCHUNK, D]),
                        op=mybir.AluOpType.mult,
                    )

                q_prep = work_pool.tile([P, NCHUNK * D], FP32, name="q_prep")
                nc.vector.tensor_scalar(
                    out=q_prep[:],
                    in0=q_tile[:],
                    scalar1=0.0,
                    scalar2=EPS,
                    op0=mybir.AluOpType.max,
                    op1=mybir.AluOpType.add,
                )

                # kv_ext [64, 65]: cols 0..63 = k^T @ v, col 64 = k^T @ ones
                kv_psum = kvps_pool.tile([D, D + 1], FP32, name="kv_psum")
                for c in range(NCHUNK):
                    ksl = k_prep[:, c * D : (c + 1) * D]
                    nc.tensor.matmul(
                        kv_psum[:, 0:D],
                        lhsT=ksl,
                        rhs=v_tile[:, c * D : (c + 1) * D],
                        start=(c == 0),
                        stop=(c == NCHUNK - 1),
                    )
                    nc.tensor.matmul(
                        kv_psum[:, D : D + 1],
                        lhsT=ksl,
                        rhs=ones_col[:],
                        start=(c == 0),
                        stop=(c == NCHUNK - 1),
                    )
                kv_sb = small_pool.tile([D, D + 1], FP32, name="kv_sb")
                nc.scalar.copy(kv_sb[:], kv_psum[:])

                # Transpose q_prep chunks into qT [64, 4096]
                qT = work_pool.tile([D, S], FP32, name="qT")
                TPC = 8  # transposes per PSUM tile ([64, 1024] = 2 banks)
                for t in range(NCHUNK // TPC):
                    tr_psum = trps_pool.tile([D, TPC * P], FP32, name="tr_psum")
                    for j in range(TPC):
                        c = t * TPC + j
                        nc.tensor.transpose(
                            tr_psum[:, j * P : (j + 1) * P],
                            in_=q_prep[:, c * D : (c + 1) * D],
                            identity=identity[:],
                        )
                    nc.scalar.copy(
                        qT[:, t * TPC * P : (t + 1) * TPC * P], tr_psum[:]
                    )

                # num chunks + divide
                out_sb = io_pool.tile([P, NCHUNK * D], FP32, name="out_sb")
                GRP = 4  # chunks per PSUM tile ([128, 260] = 1 bank)
                for g in range(NCHUNK // GRP):
                    nu_psum = nups_pool.tile([P, GRP * (D + 1)], FP32, name="nu_psum")
                    for j in range(GRP):
                        c = g * GRP + j
                        nc.tensor.matmul(
                            nu_psum[:, j * (D + 1) : (j + 1) * (D + 1)],
                            lhsT=qT[:, c * P : (c + 1) * P],
                            rhs=kv_sb[:],
                            start=True,
                            stop=True,
                        )
                    nu3 = nu_psum.rearrange("p (j e) -> p j e", e=D + 1)
                    recip = small_pool.tile([P, GRP], FP32, name="recip")
                    nc.vector.reciprocal(recip[:], nu3[:, :, D])
                    nc.vector.tensor_tensor(
                        out=out_sb.rearrange("p (n d) -> p n d", d=D)[
                            :, g * GRP : (g + 1) * GRP, :
                        ],
                        in0=nu3[:, :, 0:D],
                        in1=recip[:, :, None].to_broadcast([P, GRP, D]),
                        op=mybir.AluOpType.mult,
                    )

                nc.sync.dma_start(
                    o_d[hd].rearrange("(p n) d -> p (n d)", p=P), out_sb[:]
                )

    return nc


def kernel(query: np.ndarray, key: np.ndarray, value: np.ndarray, mask: np.ndarray) -> np.ndarray:
    global LAST_EXEC_NS
    query = np.ascontiguousarray(query, dtype=np.float32)
    key = np.ascontiguousarray(key, dtype=np.float32)
    value = np.ascontiguousarray(value, dtype=np.float32)
    mask = np.ascontiguousarray(mask, dtype=np.float32)

    use_mask = not bool(np.all(mask == 1.0))
    nc = _build_nc(use_mask)

    qf = query.reshape(B * H, S, D)
    kf = key.reshape(B * H, S, D)
    vf = value.reshape(B * H, S, D)

    in_maps = []
    for i in range(N_CORES):
        lo, hi = i * HEADS_PER_CORE, (i + 1) * HEADS_PER_CORE
        m = {
            "query": qf[lo:hi],
            "key": kf[lo:hi],
            "value": vf[lo:hi],
        }
        if use_mask:
            # head index hd -> batch (lo + hd) // H
            m["mask"] = np.stack(
                [mask[(lo + hd) // H] for hd in range(HEADS_PER_CORE)]
            )
        in_maps.append(m)

    res = run_bass_kernel_spmd(
        nc, in_maps, core_ids=list(range(N_CORES)), trace=TRACE
    )
    LAST_EXEC_NS = res.exec_time_ns

    out = np.concatenate([res.results[i]["out"] for i in range(N_CORES)], axis=0)
    return out.reshape(B, H, S, D)


# revision 10
# speedup vs baseline: 1.9216x; 1.9216x over previous
"""Performer (linear) attention kernel for Trainium2, 8-core SPMD.

Math (per batch b, head h):
    q  = relu(query) + eps
    k  = (relu(key) + eps) * mask[:, None]
    kv = k^T @ v                  # [D, D]
    ks = sum_s k                  # [D]
    num = q @ kv                  # [S, D]
    den = q @ ks                  # [S]
    out = num / den[:, None]

Sharding: 64 (b,h) heads split across 8 cores, 8 heads each. No collectives.

Per-head device plan (S=4096, D=64, P=128 partitions):
  - q/k/v loaded as [128, 2048] tiles (partition p holds rows s=32p..32p+31,
    contiguous 8KB per partition -> line-rate DMA). Chunk c = free columns
    [c*64, (c+1)*64) = the 128 rows {32p + c}.
  - DVE: k_prep/q_prep = max(x,0)+eps in one fused tensor_scalar.
  - PE:  kv[64,64] += k_chunk^T @ v_chunk, and ksum col kv[:,64] += k_chunk^T @ ones
         (32 accumulating chunk pairs into one PSUM tile [64,65]).
  - PE:  q_prep chunks transposed ([128,64]->[64,128]) into PSUM, ACT copies
         them to qT [64,4096] in SBUF.
  - PE:  num chunk [128,65] = qT_chunk^T @ kv_ext  (col 64 = denominator).
  - DVE: reciprocal of denom cols, then num * recip (broadcast) -> out tile.
  - Store out tile [128, 2048] with the same layout (one 1MB DMA).
"""

import numpy as np

from concourse import bass, mybir
import concourse.tile as tile
from concourse.masks import make_identity
from concourse.bass_utils import run_bass_kernel_spmd

B, H, S, D = 4, 16, 4096, 64
N_CORES = 8
HEADS_PER_CORE = (B * H) // N_CORES  # 8
P = 128
NCHUNK = S // P  # 32
EPS = 0.001
FP32 = mybir.dt.float32

TRACE = False
LAST_EXEC_NS = None


def _split_multi_waits(nc: bass.Bass) -> None:
    """This env's walrus codegen allows at most ONE sync wait per instruction.
    Move extra waits onto preceding single-wait NoOps on the same engine
    (per-engine program order makes this semantically identical)."""
    for _, bbh in nc.bb_map.items():
        insts = bbh.bb.instructions
        i = 0
        while i < len(insts):
            inst = insts[i]
            si = getattr(inst, "sync_info", None)
            if si is not None and si.on_wait and len(si.on_wait) > 1:
                waits = list(si.on_wait)
                for j, w in enumerate(waits[:-1]):
                    nop = mybir.InstNoOp(
                        name=f"{inst.name}-w{j}",
                        engine=inst.engine,
                        ins=[],
                        outs=[],
                        sync_info=mybir.SyncInfo(on_wait=[w], on_update=[]),
                        bass_nofuse=True,
                    )
                    insts.insert(i, nop)
                    i += 1
                inst.sync_info = mybir.SyncInfo(
                    on_wait=[waits[-1]], on_update=list(si.on_update or [])
                )
            i += 1


def _build_nc(use_mask: bool, reps: int = 1) -> bass.Bass:
    nc = bass.Bass(trn_type="TRN2")

    q_d = nc.dram_tensor("query", [HEADS_PER_CORE, S, D], FP32, kind="ExternalInput")
    k_d = nc.dram_tensor("key", [HEADS_PER_CORE, S, D], FP32, kind="ExternalInput")
    v_d = nc.dram_tensor("value", [HEADS_PER_CORE, S, D], FP32, kind="ExternalInput")
    if use_mask:
        m_d = nc.dram_tensor("mask", [HEADS_PER_CORE, S], FP32, kind="ExternalInput")
    o_d = nc.dram_tensor("out", [HEADS_PER_CORE, S, D], FP32, kind="ExternalOutput")

    with tile.TileContext(nc) as tc:
        with (
            tc.tile_pool(name="const", bufs=1) as const_pool,
            tc.tile_pool(name="io", bufs=2) as io_pool,
            tc.tile_pool(name="work", bufs=2) as work_pool,
            tc.tile_pool(name="small", bufs=2) as small_pool,
            tc.tile_pool(name="kvps", bufs=2, space="PSUM") as kvps_pool,
            tc.tile_pool(name="ksps", bufs=2, space="PSUM") as ksps_pool,
            tc.tile_pool(name="trps", bufs=2, space="PSUM") as trps_pool,
            tc.tile_pool(name="nups", bufs=2, space="PSUM") as nups_pool,
        ):
            ones_col = const_pool.tile([P, 1], FP32)
            nc.vector.memset(ones_col[:], 1.0)
            identity = const_pool.tile([P, P], FP32)
            make_identity(nc, identity[:])

            for hd in [h for _ in range(reps) for h in range(HEADS_PER_CORE)]:
                k_tile = io_pool.tile([P, NCHUNK * D], FP32, name="k_tile")
                v_tile = io_pool.tile([P, NCHUNK * D], FP32, name="v_tile")
                q_tile = io_pool.tile([P, NCHUNK * D], FP32, name="q_tile")
                nc.sync.dma_start(
                    k_tile[:], k_d[hd].rearrange("(p n) d -> p (n d)", p=P)
                )
                nc.sync.dma_start(
                    v_tile[:], v_d[hd].rearrange("(p n) d -> p (n d)", p=P)
                )
                nc.sync.dma_start(
                    q_tile[:], q_d[hd].rearrange("(p n) d -> p (n d)", p=P)
                )
                if use_mask:
                    m_tile = small_pool.tile([P, NCHUNK], FP32, name="m_tile")
                    nc.sync.dma_start(
                        m_tile[:], m_d[hd].rearrange("(p n) -> p n", p=P)
                    )

                # k_prep = max(key, 0) + eps   (then * mask if present)
                k_prep = work_pool.tile([P, NCHUNK * D], FP32, name="k_prep")
                nc.vector.tensor_scalar(
                    out=k_prep[:],
                    in0=k_tile[:],
                    scalar1=0.0,
                    scalar2=EPS,
                    op0=mybir.AluOpType.max,
                    op1=mybir.AluOpType.add,
                )
                if use_mask:
                    nc.vector.tensor_tensor(
                        out=k_prep.rearrange("p (n d) -> p n d", d=D)[:],
                        in0=k_prep.rearrange("p (n d) -> p n d", d=D)[:],
                        in1=m_tile[:, :, None].to_broadcast([P, NCHUNK, D]),
                        op=mybir.AluOpType.mult,
                    )

                q_prep = work_pool.tile([P, NCHUNK * D], FP32, name="q_prep")
                nc.vector.tensor_scalar(
                    out=q_prep[:],
                    in0=q_tile[:],
                    scalar1=0.0,
                    scalar2=EPS,
                    op0=mybir.AluOpType.max,
                    op1=mybir.AluOpType.add,
                )

                # kv [64,64] = k^T @ v and ks [64,1] = k^T @ ones, accumulated
                # in SEPARATE PSUM banks (interleaved accumulation chains in
                # one bank clobber each other at chain start).
                kv_psum = kvps_pool.tile([D, D], FP32, name="kv_psum")
                ks_psum = ksps_pool.tile([D, 1], FP32, name="ks_psum")
                for c in range(NCHUNK):
                    ksl = k_prep[:, c * D : (c + 1) * D]
                    nc.tensor.matmul(
                        ks_psum[:],
                        lhsT=ksl,
                        rhs=ones_col[:],
                        start=(c == 0),
                        stop=(c == NCHUNK - 1),
                    )
                    nc.tensor.matmul(
                        kv_psum[:],
                        lhsT=ksl,
                        rhs=v_tile[:, c * D : (c + 1) * D],
                        start=(c == 0),
                        stop=(c == NCHUNK - 1),
                    )
                # kv_sb [64, 65]: cols 0..63 = kv, col 64 = ks
                kv_sb = small_pool.tile([D, D + 1], FP32, name="kv_sb")
                nc.scalar.copy(kv_sb[:, 0:D], kv_psum[:])
                nc.scalar.copy(kv_sb[:, D : D + 1], ks_psum[:])

                # Transpose q_prep chunks into qT [64, 4096]
                qT = work_pool.tile([D, S], FP32, name="qT")
                TPC = 4  # transposes per PSUM tile ([64, 512] = 1 bank)
                for t in range(NCHUNK // TPC):
                    tr_psum = trps_pool.tile([D, TPC * P], FP32, name="tr_psum")
                    for j in range(TPC):
                        c = t * TPC + j
                        nc.tensor.transpose(
                            tr_psum[:, j * P : (j + 1) * P],
                            in_=q_prep[:, c * D : (c + 1) * D],
                            identity=identity[:],
                        )
                    nc.scalar.copy(
                        qT[:, t * TPC * P : (t + 1) * TPC * P], tr_psum[:]
                    )

                # num chunks + divide
                out_sb = io_pool.tile([P, NCHUNK * D], FP32, name="out_sb")
                GRP = 4  # chunks per PSUM tile ([128, 260] = 1 bank)
                for g in range(NCHUNK // GRP):
                    nu_psum = nups_pool.tile([P, GRP * (D + 1)], FP32, name="nu_psum")
                    for j in range(GRP):
                        c = g * GRP + j
                        nc.tensor.matmul(
                            nu_psum[:, j * (D + 1) : (j + 1) * (D + 1)],
                            lhsT=qT[:, c * P : (c + 1) * P],
                            rhs=kv_sb[:],
                            start=True,
                            stop=True,
                        )
                    nu3 = nu_psum.rearrange("p (j e) -> p j e", e=D + 1)
                    recip = small_pool.tile([P, GRP], FP32, name="recip")
                    nc.vector.reciprocal(recip[:], nu3[:, :, D])
                    nc.vector.tensor_tensor(
                        out=out_sb.rearrange("p (n d) -> p n d", d=D)[
                            :, g * GRP : (g + 1) * GRP, :
                        ],
                        in0=nu3[:, :, 0:D],
                        in1=recip[:, :, None].to_broadcast([P, GRP, D]),
                        op=mybir.AluOpType.mult,
                    )

                nc.sync.dma_start(
                    o_d[hd].rearrange("(p n) d -> p (n d)", p=P), out_sb[:]
                )

    _split_multi_waits(nc)
    return nc


def kernel(query: np.ndarray, key: np.ndarray, value: np.ndarray, mask: np.ndarray) -> np.ndarray:
    global LAST_EXEC_NS
    query = np.ascontiguousarray(query, dtype=np.float32)
    key = np.ascontiguousarray(key, dtype=np.float32)
    value = np.ascontiguousarray(value, dtype=np.float32)
    mask = np.ascontiguousarray(mask, dtype=np.float32)

    use_mask = not bool(np.all(mask == 1.0))
    nc = _build_nc(use_mask)

    qf = query.reshape(B * H, S, D)
    kf = key.reshape(B * H, S, D)
    vf = value.reshape(B * H, S, D)

    in_maps = []
    for i in range(N_CORES):
        lo, hi = i * HEADS_PER_CORE, (i + 1) * HEADS_PER_CORE
        m = {
            "query": qf[lo:hi],
            "key": kf[lo:hi],
            "value": vf[lo:hi],
        }
        if use_mask:
            # head index hd -> batch (lo + hd) // H
            m["mask"] = np.stack(
                [mask[(lo + hd) // H] for hd in range(HEADS_PER_CORE)]
            )
        in_maps.append(m)

    res = run_bass_kernel_spmd(
        nc, in_maps, core_ids=list(range(N_CORES)), trace=TRACE
    )
    LAST_EXEC_NS = res.exec_time_ns

    out = np.concatenate([res.results[i]["out"] for i in range(N_CORES)], axis=0)
    return out.reshape(B, H, S, D)
